# revision 36
# baseline (speedup 1.0000x reference)
"""GATv2 layer (broadcast-score variant) as a Bass/Tile kernel on 8 NeuronCores.

Math: since scores[i,j] = e[j] (row-broadcast) masked by A, the masked softmax +
aggregation collapse to
    g = exp(e),  e = relu(X @ W.T) @ a_w
    out = relu( (A @ (g*Wh)) / (A @ g) )          with Wh = X @ W.T
Each core computes a 1024-row block of the output.

v5: phase-2 in fp8e4 + DoubleRow over 32 chunks of 256 j-rows (A is exactly
0/1 in fp8; G=[g*Wh|g] fp8-quantized), interleaved chunk-by-chunk with
phase-1 in PE program order so the PE never idles. Phase-1 elementwise work is
spread at ~1 instruction each over Act (relu, 2x exp), DVE (reduce, fused
broadcast G-scale), gpsimd (m4) -- instruction COUNT, not element count,
dominates the elementwise engines.
"""

import os

import numpy as np

import concourse.tile as tile
from concourse import bacc, mybir
from concourse.bass_utils import run_bass_kernel_spmd

N, IN_DIM, OUT_DIM = 8192, 256, 128
NCORES = 8
RPC = N // NCORES          # rows per core (1024)
P = 128                    # partitions
DH = IN_DIM // P           # 2 chunks of the d-contraction
NC2 = N // 256             # 32 double-row chunks (256 j-rows each)
HF = RPC // 2              # 512-wide i-halves for phase-2 matmuls
QW = 256                   # epilogue quarter width

F32 = mybir.dt.float32
BF16 = mybir.dt.bfloat16
F8 = mybir.dt.float8e4
AFT = mybir.ActivationFunctionType
ALU = mybir.AluOpType
DR = mybir.MatmulPerfMode.DoubleRow
GP = 144                   # G row pitch in fp8 bytes (129 used; 16B aligned)

ATBUFS = int(os.environ.get("ATBUFS", "8"))
XPIECES = 8                # xt DMA split (lazy-issued inside the loop)
M4_ENG = os.environ.get("M4_ENG", "gpsimd")
F8E3 = mybir.dt.float8e3
XT_DT = {"bf16": BF16, "f8": F8, "f8e3": F8E3}[os.environ.get("XT_DT", "f8e3")]
ATSPLIT = int(os.environ.get("ATSPLIT", "2"))
PH1_LEAD = int(os.environ.get("PH1_LEAD", "2"))  # phase-1 iters ahead of phase-2


def emit_body(nc, tc, io, pools):
    at, xt, wt, awr, out = io
    big, atp, ph1, outp = pools

    # persistent SBUF tensors
    xt_sb = big.tile([P, DH, N], XT_DT, tag="xt_sb", name="xt_sb")
    wt_sb = big.tile([P, DH, OUT_DIM], BF16, tag="wt_sb", name="wt_sb")
    aw_sb = big.tile([P, 2, OUT_DIM], BF16, tag="aw_sb", name="aw_sb")
    G = big.tile([P, NC2, 2, GP], F8, tag="G", name="G")
    ones = big.tile([1, P], F32, tag="ones", name="ones")
    nc.vector.memset(ones, 1.0)

    nc.sync.dma_start(out=wt_sb, in_=wt.rearrange("(p dh) o -> p dh o", p=P))
    nc.sync.dma_start(out=aw_sb, in_=awr.rearrange("p (b o) -> p b o", b=2))
    xp = N // XPIECES
    xt_r = xt.rearrange("(p dh) n -> p dh n", p=P)

    def issue_xt_piece(x):
        nc.scalar.dma_start(
            out=xt_sb[:, :, x * xp:(x + 1) * xp],
            in_=xt_r[:, :, x * xp:(x + 1) * xp],
        )

    issue_xt_piece(0)
    issue_xt_piece(1)
    issue_xt_piece(2)

    m4_eng = nc.gpsimd if M4_ENG == "gpsimd" else nc.vector

    with tc.tile_pool(name="ps", bufs=1, space="PSUM") as ps:
        nm = [ps.tile([P, HF], F32, tag=f"nm{h}", name=f"nm{h}", bufs=1)
              for h in range(2)]
        dn = [ps.tile([1, HF], F32, tag=f"dn{h}", name=f"dn{h}", bufs=1)
              for h in range(2)]

        def emit_ph2(it, at_tile):
            # ---- phase 2 chunk it: fp8 DoubleRow, 256-deep contraction ----
            # dn before nm so the final chunk's dn stops first and the
            # epilogue reciprocal can overlap the last nm matmuls
            for h in range(2):
                nc.tensor.matmul(
                    dn[h][0:1, :],
                    G[:, it, :, OUT_DIM:OUT_DIM + 1],
                    at_tile[:, :, h * HF:(h + 1) * HF],
                    start=(it == 0),
                    stop=(it == NC2 - 1),
                    perf_mode=DR,
                )
            for h in range(2):
                nc.tensor.matmul(
                    nm[h][:, :],
                    G[:, it, :, 0:OUT_DIM],
                    at_tile[:, :, h * HF:(h + 1) * HF],
                    start=(it == 0),
                    stop=(it == NC2 - 1),
                    perf_mode=DR,
                )

        at_tiles = {}
        for it in range(NC2):
            # lazy xt pieces, three ahead of the consuming iteration
            if it % (NC2 // XPIECES) == 0:
                nx = it // (NC2 // XPIECES) + 3
                if nx < XPIECES:
                    issue_xt_piece(nx)
            # ---- phase-2 input: A.T chunk [128, 2, 1024] fp8 (host-packed) --
            # two dma_starts per chunk to engage more DMA queues in parallel
            at_sb = atp.tile([P, 2, RPC], F8, tag="at_sb", name="at_sb")
            at_r = at[it].rearrange("p (k s i) -> p k s i", k=2, s=ATSPLIT // 2)
            at_w = at_sb.rearrange("p k (s i) -> p k s i", s=ATSPLIT // 2)
            for k in range(2):
                for s in range(ATSPLIT // 2):
                    nc.sync.dma_start(out=at_w[:, k, s, :], in_=at_r[:, k, s, :])
            at_tiles[it] = at_sb

            # ---- phase 1, j-tiles t=2it, 2it+1: Wh -> e -> g -> G chunk ----
            wh4 = ps.tile([P, 2, OUT_DIM], F32, tag="wh4", name="wh4", bufs=3)
            for k in range(2):
                t = 2 * it + k
                for dh in range(DH):
                    nc.tensor.matmul(
                        wh4[:, k, :],
                        xt_sb[:, dh, t * P:(t + 1) * P],
                        wt_sb[:, dh, :],
                        start=(dh == 0),
                        stop=(dh == DH - 1),
                    )
            r4 = ph1.tile([P, 2, OUT_DIM], BF16, name="r4")
            nc.scalar.activation(r4, wh4, AFT.Relu)                 # Act 1
            m4 = ph1.tile([P, 2, OUT_DIM], BF16, name="m4")
            m4_eng.tensor_mul(m4, r4, aw_sb)                        # gpsimd 1
            e4 = ph1.tile([P, 2], F32, name="e4")
            nc.vector.reduce_sum(e4, m4, axis=mybir.AxisListType.X)  # DVE 1
            g4 = ph1.tile([P, 2], F32, name="g4")
            nc.scalar.activation(g4, e4, AFT.Exp)                   # Act 2
            nc.scalar.activation(
                G[:, it, :, OUT_DIM:OUT_DIM + 1], e4, AFT.Exp,      # Act 3
            )
            # fused G[:, :, 0:128] = g * Wh via stride-0 broadcast  # DVE 2
            g_bc = g4.unsqueeze(2).broadcast_to([P, 2, OUT_DIM])
            nc.vector.tensor_tensor(
                out=G[:, it, :, 0:OUT_DIM], in0=wh4, in1=g_bc, op=ALU.mult,
            )

            # phase-2 trails phase-1 by PH1_LEAD chunks so G's DoubleRow
            # LDWEIGHTS dependency is satisfied long before PE reaches it
            if it >= PH1_LEAD:
                emit_ph2(it - PH1_LEAD, at_tiles.pop(it - PH1_LEAD))
        for it in range(NC2 - PH1_LEAD, NC2):
            emit_ph2(it, at_tiles.pop(it))

        # ---- epilogue: out[o, i] = relu(nm)/dn, pipelined 512-col halves ----
        for h in range(2):
            rc1 = outp.tile([1, HF], F32, tag="rc1", name="rc1")
            nc.vector.reciprocal(rc1, dn[h][0:1, :])
            dbc = ps.tile([P, HF], F32, tag="dbc", name="dbc", bufs=1)
            nc.tensor.matmul(dbc, ones[0:1, 0:P], rc1,
                             start=True, stop=True)
            rel = outp.tile([P, HF], F32, tag="rel", name="rel")
            nc.scalar.activation(rel, nm[h], AFT.Relu)
            o_sb = outp.tile([P, HF], F32, tag="osb", name="osb")
            nc.vector.tensor_mul(o_sb, rel, dbc)
            nc.sync.dma_start(out=out[:, h * HF:(h + 1) * HF], in_=o_sb)


def build_nc(repeat=1):
    nc = bacc.Bacc("TRN2", target_bir_lowering=False)
    # at: A.T col-block, packed [NC2, 128, 2*1024] fp8 (j = 256c + 128k + p)
    at = nc.dram_tensor("at", [NC2, P, 2 * RPC], F8, kind="ExternalInput")
    # xt: X.T packed p-major [128, 2, 8192] (din = dh*128 + p)
    xt = nc.dram_tensor("xt", [P * DH, N], XT_DT, kind="ExternalInput")
    # wt: W.T packed p-major [128, 2, 128] bf16
    wt = nc.dram_tensor("wt", [P * DH, OUT_DIM], BF16, kind="ExternalInput")
    awr = nc.dram_tensor("awr", [P, 2 * OUT_DIM], BF16, kind="ExternalInput")
    out = nc.dram_tensor("out", [OUT_DIM, RPC], F32, kind="ExternalOutput")

    with tile.TileContext(nc) as tc:
        with (
            tc.tile_pool(name="big", bufs=1) as big,
            tc.tile_pool(name="atp", bufs=ATBUFS) as atp,
            tc.tile_pool(name="ph1", bufs=4) as ph1,
            tc.tile_pool(name="outp", bufs=2) as outp,
        ):
            for _ in range(repeat):
                emit_body(nc, tc, (at, xt, wt, awr, out), (big, atp, ph1, outp))
    nc.compile()
    return nc


_NC_CACHE = None


def _get_nc():
    global _NC_CACHE
    if _NC_CACHE is None:
        _NC_CACHE = build_nc()
    return _NC_CACHE


def make_in_maps(X, A, W, a_w):
    X = np.ascontiguousarray(np.asarray(X, dtype=np.float32))
    A = np.ascontiguousarray(np.asarray(A, dtype=np.float32))
    W = np.ascontiguousarray(np.asarray(W, dtype=np.float32))
    a_w = np.ascontiguousarray(np.asarray(a_w, dtype=np.float32))

    bf16 = mybir.dt.np(BF16)
    f8 = mybir.dt.np(F8)

    xt_full = X.T.astype(mybir.dt.np(XT_DT))   # [256, 8192], row = din
    xt = np.ascontiguousarray(
        xt_full.reshape(DH, P, N).transpose(1, 0, 2).reshape(P * DH, N)
    )
    wt_full = W.T.astype(bf16)                 # [256, 128]
    wt = np.ascontiguousarray(
        wt_full.reshape(DH, P, OUT_DIM).transpose(1, 0, 2).reshape(P * DH, OUT_DIM)
    )
    awr = np.ascontiguousarray(
        np.broadcast_to(np.tile(a_w, 2)[None, :], (P, 2 * OUT_DIM)).astype(bf16)
    )

    in_maps = []
    for c in range(NCORES):
        atb = A[c * RPC:(c + 1) * RPC, :].T.astype(f8)   # [8192, 1024]
        # pack [N, RPC] -> [NC2, 128, 2*1024]: j = 256*cc + 128*k + p
        atp_np = np.ascontiguousarray(
            atb.reshape(NC2, 2, P, RPC).transpose(0, 2, 1, 3).reshape(NC2, P, 2 * RPC)
        )
        in_maps.append({"at": atp_np, "xt": xt, "wt": wt, "awr": awr})
    return in_maps


def kernel_with_results(X, A, W, a_w, trace=False):
    in_maps = make_in_maps(X, A, W, a_w)
    res = run_bass_kernel_spmd(_get_nc(), in_maps, list(range(NCORES)), trace=trace)
    out = np.concatenate(
        [np.ascontiguousarray(r["out"].T) for r in res.results], axis=0
    )
    return out.astype(np.float32), res


def kernel(X, A, W, a_w):
    out, _ = kernel_with_results(X, A, W, a_w)
    return out


# revision 41
# speedup vs baseline: 1.1529x; 1.1529x over previous
"""GATv2 layer (broadcast-score variant) as a Bass/Tile kernel on 8 NeuronCores.

Math: since scores[i,j] = e[j] (row-broadcast) masked by A, the masked softmax +
aggregation collapse to
    g = exp(e),  e = relu(X @ W.T) @ a_w
    out = relu( (A @ (g*Wh)) / (A @ g) )          with Wh = X @ W.T
Each core computes a 1024-row block of the output.

v5: phase-2 in fp8e4 + DoubleRow over 32 chunks of 256 j-rows (A is exactly
0/1 in fp8; G=[g*Wh|g] fp8-quantized), interleaved chunk-by-chunk with
phase-1 in PE program order so the PE never idles. Phase-1 elementwise work is
spread at ~1 instruction each over Act (relu, 2x exp), DVE (reduce, fused
broadcast G-scale), gpsimd (m4) -- instruction COUNT, not element count,
dominates the elementwise engines.
"""

import os

import numpy as np

import concourse.tile as tile
from concourse import bacc, mybir
from concourse.bass_utils import run_bass_kernel_spmd

N, IN_DIM, OUT_DIM = 8192, 256, 128
NCORES = 8
RPC = N // NCORES          # rows per core (1024)
P = 128                    # partitions
DH = IN_DIM // P           # 2 chunks of the d-contraction
NC2 = N // 256             # 32 double-row chunks (256 j-rows each)
HF = RPC // 2              # 512-wide i-halves for phase-2 matmuls
QW = 256                   # epilogue quarter width

F32 = mybir.dt.float32
BF16 = mybir.dt.bfloat16
F8 = mybir.dt.float8e4
AFT = mybir.ActivationFunctionType
ALU = mybir.AluOpType
DR = mybir.MatmulPerfMode.DoubleRow
GP = 144                   # G row pitch in fp8 bytes (129 used; 16B aligned)

ATBUFS = int(os.environ.get("ATBUFS", "12"))
XPIECES = 16               # xt DMA split (lazy-issued inside the loop)
M4_ENG = os.environ.get("M4_ENG", "gpsimd")
F8E3 = mybir.dt.float8e3
XT_DT = {"bf16": BF16, "f8": F8, "f8e3": F8E3}[os.environ.get("XT_DT", "f8e3")]
ATSPLIT = int(os.environ.get("ATSPLIT", "2"))
PH1_LEAD = int(os.environ.get("PH1_LEAD", "2"))  # phase-1 iters ahead of phase-2


def emit_body(nc, tc, io, pools):
    at, xt, wt, awr, out = io
    big, atp, ph1, outp = pools

    # persistent SBUF tensors
    xt_sb = big.tile([P, DH, N], XT_DT, tag="xt_sb", name="xt_sb")
    wt_sb = big.tile([P, DH, OUT_DIM], BF16, tag="wt_sb", name="wt_sb")
    aw_sb = big.tile([P, 2, OUT_DIM], BF16, tag="aw_sb", name="aw_sb")
    G = big.tile([P, NC2, 2, GP], F8, tag="G", name="G")
    ones = big.tile([1, P], F32, tag="ones", name="ones")
    nc.vector.memset(ones, 1.0)

    nc.sync.dma_start(out=wt_sb, in_=wt.rearrange("(p dh) o -> p dh o", p=P))
    nc.sync.dma_start(out=aw_sb, in_=awr.rearrange("p (b o) -> p b o", b=2))
    xp = N // XPIECES
    xt_r = xt.rearrange("(p dh) n -> p dh n", p=P)

    def issue_xt_piece(x):
        nc.scalar.dma_start(
            out=xt_sb[:, :, x * xp:(x + 1) * xp],
            in_=xt_r[:, :, x * xp:(x + 1) * xp],
        )

    for _x in range(4):
        issue_xt_piece(_x)

    m4_eng = nc.gpsimd if M4_ENG == "gpsimd" else nc.vector

    with tc.tile_pool(name="ps", bufs=1, space="PSUM") as ps:
        nm = [ps.tile([P, HF], F32, tag=f"nm{h}", name=f"nm{h}", bufs=1)
              for h in range(2)]
        dn = [ps.tile([1, HF], F32, tag=f"dn{h}", name=f"dn{h}", bufs=1)
              for h in range(2)]

        def emit_ph2(it, at_tile):
            # ---- phase 2 chunk it: fp8 DoubleRow, 256-deep contraction ----
            # dn before nm so the final chunk's dn stops first and the
            # epilogue reciprocal can overlap the last nm matmuls
            for h in range(2):
                nc.tensor.matmul(
                    dn[h][0:1, :],
                    G[:, it, :, OUT_DIM:OUT_DIM + 1],
                    at_tile[:, :, h * HF:(h + 1) * HF],
                    start=(it == 0),
                    stop=(it == NC2 - 1),
                    perf_mode=DR,
                )
            for h in range(2):
                nc.tensor.matmul(
                    nm[h][:, :],
                    G[:, it, :, 0:OUT_DIM],
                    at_tile[:, :, h * HF:(h + 1) * HF],
                    start=(it == 0),
                    stop=(it == NC2 - 1),
                    perf_mode=DR,
                )

        at_tiles = {}
        for it in range(NC2):
            # lazy xt pieces, four ahead of the consuming iteration
            if it % (NC2 // XPIECES) == 0:
                nx = it // (NC2 // XPIECES) + 4
                if nx < XPIECES:
                    issue_xt_piece(nx)
            # ---- phase-2 input: A.T chunk [128, 2, 1024] fp8 (host-packed) --
            # two dma_starts per chunk to engage more DMA queues in parallel
            at_sb = atp.tile([P, 2, RPC], F8, tag="at_sb", name="at_sb")
            at_r = at[it].rearrange("p (k s i) -> p k s i", k=2, s=ATSPLIT // 2)
            at_w = at_sb.rearrange("p k (s i) -> p k s i", s=ATSPLIT // 2)
            for k in range(2):
                for s in range(ATSPLIT // 2):
                    nc.sync.dma_start(out=at_w[:, k, s, :], in_=at_r[:, k, s, :])
            at_tiles[it] = at_sb

            # ---- phase 1, j-tiles t=2it, 2it+1: Wh -> e -> g -> G chunk ----
            wh4 = ps.tile([P, 2, OUT_DIM], F32, tag="wh4", name="wh4", bufs=3)
            for k in range(2):
                t = 2 * it + k
                for dh in range(DH):
                    nc.tensor.matmul(
                        wh4[:, k, :],
                        xt_sb[:, dh, t * P:(t + 1) * P],
                        wt_sb[:, dh, :],
                        start=(dh == 0),
                        stop=(dh == DH - 1),
                    )
            r4 = ph1.tile([P, 2, OUT_DIM], BF16, name="r4")
            nc.scalar.activation(r4, wh4, AFT.Relu)                 # Act 1
            m4 = ph1.tile([P, 2, OUT_DIM], BF16, name="m4")
            m4_eng.tensor_mul(m4, r4, aw_sb)                        # gpsimd 1
            e4 = ph1.tile([P, 2], F32, name="e4")
            nc.vector.reduce_sum(e4, m4, axis=mybir.AxisListType.X)  # DVE 1
            g4 = ph1.tile([P, 2], F32, name="g4")
            nc.scalar.activation(g4, e4, AFT.Exp)                   # Act 2
            nc.scalar.activation(
                G[:, it, :, OUT_DIM:OUT_DIM + 1], e4, AFT.Exp,      # Act 3
            )
            # fused G[:, :, 0:128] = g * Wh via stride-0 broadcast  # DVE 2
            g_bc = g4.unsqueeze(2).broadcast_to([P, 2, OUT_DIM])
            nc.vector.tensor_tensor(
                out=G[:, it, :, 0:OUT_DIM], in0=wh4, in1=g_bc, op=ALU.mult,
            )

            # phase-2 trails phase-1 by PH1_LEAD chunks so G's DoubleRow
            # LDWEIGHTS dependency is satisfied long before PE reaches it
            if it >= PH1_LEAD:
                emit_ph2(it - PH1_LEAD, at_tiles.pop(it - PH1_LEAD))
        for it in range(NC2 - PH1_LEAD, NC2):
            emit_ph2(it, at_tiles.pop(it))

        # ---- epilogue: out[o, i] = relu(nm)/dn, pipelined 512-col halves ----
        for h in range(2):
            rc1 = outp.tile([1, HF], F32, tag="rc1", name="rc1")
            nc.vector.reciprocal(rc1, dn[h][0:1, :])
            dbc = ps.tile([P, HF], F32, tag="dbc", name="dbc", bufs=1)
            nc.tensor.matmul(dbc, ones[0:1, 0:P], rc1,
                             start=True, stop=True)
            rel = outp.tile([P, HF], F32, tag="rel", name="rel")
            nc.scalar.activation(rel, nm[h], AFT.Relu)
            o_sb = outp.tile([P, HF], F32, tag="osb", name="osb")
            nc.vector.tensor_mul(o_sb, rel, dbc)
            nc.sync.dma_start(out=out[:, h * HF:(h + 1) * HF], in_=o_sb)


def build_nc(repeat=1):
    nc = bacc.Bacc("TRN2", target_bir_lowering=False)
    # at: A.T col-block, packed [NC2, 128, 2*1024] fp8 (j = 256c + 128k + p)
    at = nc.dram_tensor("at", [NC2, P, 2 * RPC], F8, kind="ExternalInput")
    # xt: X.T packed p-major [128, 2, 8192] (din = dh*128 + p)
    xt = nc.dram_tensor("xt", [P * DH, N], XT_DT, kind="ExternalInput")
    # wt: W.T packed p-major [128, 2, 128] bf16
    wt = nc.dram_tensor("wt", [P * DH, OUT_DIM], BF16, kind="ExternalInput")
    awr = nc.dram_tensor("awr", [P, 2 * OUT_DIM], BF16, kind="ExternalInput")
    out = nc.dram_tensor("out", [OUT_DIM, RPC], F32, kind="ExternalOutput")

    with tile.TileContext(nc) as tc:
        with (
            tc.tile_pool(name="big", bufs=1) as big,
            tc.tile_pool(name="atp", bufs=ATBUFS) as atp,
            tc.tile_pool(name="ph1", bufs=4) as ph1,
            tc.tile_pool(name="outp", bufs=2) as outp,
        ):
            for _ in range(repeat):
                emit_body(nc, tc, (at, xt, wt, awr, out), (big, atp, ph1, outp))
    nc.compile()
    return nc


_NC_CACHE = None


def _get_nc():
    global _NC_CACHE
    if _NC_CACHE is None:
        _NC_CACHE = build_nc()
    return _NC_CACHE


def make_in_maps(X, A, W, a_w):
    X = np.ascontiguousarray(np.asarray(X, dtype=np.float32))
    A = np.ascontiguousarray(np.asarray(A, dtype=np.float32))
    W = np.ascontiguousarray(np.asarray(W, dtype=np.float32))
    a_w = np.ascontiguousarray(np.asarray(a_w, dtype=np.float32))

    bf16 = mybir.dt.np(BF16)
    f8 = mybir.dt.np(F8)

    xt_full = X.T.astype(mybir.dt.np(XT_DT))   # [256, 8192], row = din
    xt = np.ascontiguousarray(
        xt_full.reshape(DH, P, N).transpose(1, 0, 2).reshape(P * DH, N)
    )
    wt_full = W.T.astype(bf16)                 # [256, 128]
    wt = np.ascontiguousarray(
        wt_full.reshape(DH, P, OUT_DIM).transpose(1, 0, 2).reshape(P * DH, OUT_DIM)
    )
    awr = np.ascontiguousarray(
        np.broadcast_to(np.tile(a_w, 2)[None, :], (P, 2 * OUT_DIM)).astype(bf16)
    )

    in_maps = []
    for c in range(NCORES):
        atb = A[c * RPC:(c + 1) * RPC, :].T.astype(f8)   # [8192, 1024]
        # pack [N, RPC] -> [NC2, 128, 2*1024]: j = 256*cc + 128*k + p
        atp_np = np.ascontiguousarray(
            atb.reshape(NC2, 2, P, RPC).transpose(0, 2, 1, 3).reshape(NC2, P, 2 * RPC)
        )
        in_maps.append({"at": atp_np, "xt": xt, "wt": wt, "awr": awr})
    return in_maps


def kernel_with_results(X, A, W, a_w, trace=False):
    in_maps = make_in_maps(X, A, W, a_w)
    res = run_bass_kernel_spmd(_get_nc(), in_maps, list(range(NCORES)), trace=trace)
    out = np.concatenate(
        [np.ascontiguousarray(r["out"].T) for r in res.results], axis=0
    )
    return out.astype(np.float32), res


def kernel(X, A, W, a_w):
    out, _ = kernel_with_results(X, A, W, a_w)
    return out


# revision 42
# speedup vs baseline: 1.2300x; 1.0669x over previous
"""GATv2 layer (broadcast-score variant) as a Bass/Tile kernel on 8 NeuronCores.

Math: since scores[i,j] = e[j] (row-broadcast) masked by A, the masked softmax +
aggregation collapse to
    g = exp(e),  e = relu(X @ W.T) @ a_w
    out = relu( (A @ (g*Wh)) / (A @ g) )          with Wh = X @ W.T
Each core computes a 1024-row block of the output.

v5: phase-2 in fp8e4 + DoubleRow over 32 chunks of 256 j-rows (A is exactly
0/1 in fp8; G=[g*Wh|g] fp8-quantized), interleaved chunk-by-chunk with
phase-1 in PE program order so the PE never idles. Phase-1 elementwise work is
spread at ~1 instruction each over Act (relu, 2x exp), DVE (reduce, fused
broadcast G-scale), gpsimd (m4) -- instruction COUNT, not element count,
dominates the elementwise engines.
"""

import os

import numpy as np

import concourse.tile as tile
from concourse import bacc, mybir
from concourse.bass_utils import run_bass_kernel_spmd

N, IN_DIM, OUT_DIM = 8192, 256, 128
NCORES = 8
RPC = N // NCORES          # rows per core (1024)
P = 128                    # partitions
DH = IN_DIM // P           # 2 chunks of the d-contraction
NC2 = N // 256             # 32 double-row chunks (256 j-rows each)
HF = RPC // 2              # 512-wide i-halves for phase-2 matmuls
QW = 256                   # epilogue quarter width

F32 = mybir.dt.float32
BF16 = mybir.dt.bfloat16
F8 = mybir.dt.float8e4
AFT = mybir.ActivationFunctionType
ALU = mybir.AluOpType
DR = mybir.MatmulPerfMode.DoubleRow
GP = 144                   # G row pitch in fp8 bytes (129 used; 16B aligned)

ATBUFS = int(os.environ.get("ATBUFS", "12"))
XPIECES = 8                # xt DMA split (lazy-issued inside the loop)
M4_ENG = os.environ.get("M4_ENG", "gpsimd")
F8E3 = mybir.dt.float8e3
XT_DT = {"bf16": BF16, "f8": F8, "f8e3": F8E3}[os.environ.get("XT_DT", "f8e3")]
ATSPLIT = int(os.environ.get("ATSPLIT", "2"))
PH1_LEAD = int(os.environ.get("PH1_LEAD", "2"))  # phase-1 iters ahead of phase-2


def emit_body(nc, tc, io, pools):
    at, xt, wt, awr, out = io
    big, atp, ph1, outp = pools

    # persistent SBUF tensors
    xt_sb = big.tile([P, DH, N], XT_DT, tag="xt_sb", name="xt_sb")
    wt_sb = big.tile([P, DH, OUT_DIM], BF16, tag="wt_sb", name="wt_sb")
    aw_sb = big.tile([P, 2, OUT_DIM], BF16, tag="aw_sb", name="aw_sb")
    G = big.tile([P, NC2, 2, GP], F8, tag="G", name="G")
    ones = big.tile([1, P], F32, tag="ones", name="ones")
    nc.vector.memset(ones, 1.0)

    nc.sync.dma_start(out=wt_sb, in_=wt.rearrange("(p dh) o -> p dh o", p=P))
    nc.sync.dma_start(out=aw_sb, in_=awr.rearrange("p (b o) -> p b o", b=2))
    xp = N // XPIECES
    xt_r = xt.rearrange("(p dh) n -> p dh n", p=P)

    def issue_xt_piece(x):
        nc.scalar.dma_start(
            out=xt_sb[:, :, x * xp:(x + 1) * xp],
            in_=xt_r[:, :, x * xp:(x + 1) * xp],
        )

    issue_xt_piece(0)
    issue_xt_piece(1)

    m4_eng = nc.gpsimd if M4_ENG == "gpsimd" else nc.vector

    with tc.tile_pool(name="ps", bufs=1, space="PSUM") as ps:
        nm = [ps.tile([P, HF], F32, tag=f"nm{h}", name=f"nm{h}", bufs=1)
              for h in range(2)]
        dn = [ps.tile([1, HF], F32, tag=f"dn{h}", name=f"dn{h}", bufs=1)
              for h in range(2)]

        def emit_ph2(it, at_tile):
            # ---- phase 2 chunk it: fp8 DoubleRow, 256-deep contraction ----
            # dn before nm so the final chunk's dn stops first and the
            # epilogue reciprocal can overlap the last nm matmuls
            for h in range(2):
                nc.tensor.matmul(
                    dn[h][0:1, :],
                    G[:, it, :, OUT_DIM:OUT_DIM + 1],
                    at_tile[:, :, h * HF:(h + 1) * HF],
                    start=(it == 0),
                    stop=(it == NC2 - 1),
                    perf_mode=DR,
                )
            for h in range(2):
                nc.tensor.matmul(
                    nm[h][:, :],
                    G[:, it, :, 0:OUT_DIM],
                    at_tile[:, :, h * HF:(h + 1) * HF],
                    start=(it == 0),
                    stop=(it == NC2 - 1),
                    perf_mode=DR,
                )

        at_tiles = {}
        for it in range(NC2):
            # lazy xt pieces, two ahead of the consuming iteration
            if it % (NC2 // XPIECES) == 0:
                nx = it // (NC2 // XPIECES) + 2
                if nx < XPIECES:
                    issue_xt_piece(nx)
            # ---- phase-2 input: A.T chunk [128, 2, 1024] fp8 (host-packed) --
            # two dma_starts per chunk to engage more DMA queues in parallel
            at_sb = atp.tile([P, 2, RPC], F8, tag="at_sb", name="at_sb")
            at_r = at[it].rearrange("p (k s i) -> p k s i", k=2, s=ATSPLIT // 2)
            at_w = at_sb.rearrange("p k (s i) -> p k s i", s=ATSPLIT // 2)
            for k in range(2):
                for s in range(ATSPLIT // 2):
                    nc.sync.dma_start(out=at_w[:, k, s, :], in_=at_r[:, k, s, :])
            at_tiles[it] = at_sb

            # ---- phase 1, j-tiles t=2it, 2it+1: Wh -> e -> g -> G chunk ----
            wh4 = ps.tile([P, 2, OUT_DIM], F32, tag="wh4", name="wh4", bufs=3)
            for k in range(2):
                t = 2 * it + k
                for dh in range(DH):
                    nc.tensor.matmul(
                        wh4[:, k, :],
                        xt_sb[:, dh, t * P:(t + 1) * P],
                        wt_sb[:, dh, :],
                        start=(dh == 0),
                        stop=(dh == DH - 1),
                    )
            r4 = ph1.tile([P, 2, OUT_DIM], BF16, name="r4")
            nc.scalar.activation(r4, wh4, AFT.Relu)                 # Act 1
            m4 = ph1.tile([P, 2, OUT_DIM], BF16, name="m4")
            m4_eng.tensor_mul(m4, r4, aw_sb)                        # gpsimd 1
            e4 = ph1.tile([P, 2], F32, name="e4")
            nc.vector.reduce_sum(e4, m4, axis=mybir.AxisListType.X)  # DVE 1
            g4 = ph1.tile([P, 2], F32, name="g4")
            nc.scalar.activation(g4, e4, AFT.Exp)                   # Act 2
            nc.scalar.activation(
                G[:, it, :, OUT_DIM:OUT_DIM + 1], e4, AFT.Exp,      # Act 3
            )
            # fused G[:, :, 0:128] = g * Wh via stride-0 broadcast  # DVE 2
            g_bc = g4.unsqueeze(2).broadcast_to([P, 2, OUT_DIM])
            nc.vector.tensor_tensor(
                out=G[:, it, :, 0:OUT_DIM], in0=wh4, in1=g_bc, op=ALU.mult,
            )

            # phase-2 trails phase-1 by PH1_LEAD chunks so G's DoubleRow
            # LDWEIGHTS dependency is satisfied long before PE reaches it
            if it >= PH1_LEAD:
                emit_ph2(it - PH1_LEAD, at_tiles.pop(it - PH1_LEAD))
        for it in range(NC2 - PH1_LEAD, NC2):
            emit_ph2(it, at_tiles.pop(it))

        # ---- epilogue: out[o, i] = relu(nm)/dn, pipelined 256-col quarters --
        for h in range(2):
            for q in range(2):
                sl = slice(q * QW, (q + 1) * QW)
                rc1 = outp.tile([1, QW], F32, tag="rc1", name="rc1")
                nc.vector.reciprocal(rc1, dn[h][0:1, sl])
                dbc = ps.tile([P, QW], F32, tag="dbc", name="dbc", bufs=1)
                nc.tensor.matmul(dbc, ones[0:1, 0:P], rc1,
                                 start=True, stop=True)
                rel = outp.tile([P, QW], F32, tag="rel", name="rel")
                nc.scalar.activation(rel, nm[h][:, sl], AFT.Relu)
                o_sb = outp.tile([P, QW], F32, tag="osb", name="osb")
                nc.vector.tensor_mul(o_sb, rel, dbc)
                nc.sync.dma_start(out=out[:, h * HF + q * QW:h * HF + (q + 1) * QW],
                                  in_=o_sb)


def build_nc(repeat=1):
    nc = bacc.Bacc("TRN2", target_bir_lowering=False)
    # at: A.T col-block, packed [NC2, 128, 2*1024] fp8 (j = 256c + 128k + p)
    at = nc.dram_tensor("at", [NC2, P, 2 * RPC], F8, kind="ExternalInput")
    # xt: X.T packed p-major [128, 2, 8192] (din = dh*128 + p)
    xt = nc.dram_tensor("xt", [P * DH, N], XT_DT, kind="ExternalInput")
    # wt: W.T packed p-major [128, 2, 128] bf16
    wt = nc.dram_tensor("wt", [P * DH, OUT_DIM], BF16, kind="ExternalInput")
    awr = nc.dram_tensor("awr", [P, 2 * OUT_DIM], BF16, kind="ExternalInput")
    out = nc.dram_tensor("out", [OUT_DIM, RPC], F32, kind="ExternalOutput")

    with tile.TileContext(nc) as tc:
        with (
            tc.tile_pool(name="big", bufs=1) as big,
            tc.tile_pool(name="atp", bufs=ATBUFS) as atp,
            tc.tile_pool(name="ph1", bufs=4) as ph1,
            tc.tile_pool(name="outp", bufs=2) as outp,
        ):
            for _ in range(repeat):
                emit_body(nc, tc, (at, xt, wt, awr, out), (big, atp, ph1, outp))
    nc.compile()
    return nc


_NC_CACHE = None


def _get_nc():
    global _NC_CACHE
    if _NC_CACHE is None:
        _NC_CACHE = build_nc()
    return _NC_CACHE


def make_in_maps(X, A, W, a_w):
    X = np.ascontiguousarray(np.asarray(X, dtype=np.float32))
    A = np.ascontiguousarray(np.asarray(A, dtype=np.float32))
    W = np.ascontiguousarray(np.asarray(W, dtype=np.float32))
    a_w = np.ascontiguousarray(np.asarray(a_w, dtype=np.float32))

    bf16 = mybir.dt.np(BF16)
    f8 = mybir.dt.np(F8)

    xt_full = X.T.astype(mybir.dt.np(XT_DT))   # [256, 8192], row = din
    xt = np.ascontiguousarray(
        xt_full.reshape(DH, P, N).transpose(1, 0, 2).reshape(P * DH, N)
    )
    wt_full = W.T.astype(bf16)                 # [256, 128]
    wt = np.ascontiguousarray(
        wt_full.reshape(DH, P, OUT_DIM).transpose(1, 0, 2).reshape(P * DH, OUT_DIM)
    )
    awr = np.ascontiguousarray(
        np.broadcast_to(np.tile(a_w, 2)[None, :], (P, 2 * OUT_DIM)).astype(bf16)
    )

    in_maps = []
    for c in range(NCORES):
        atb = A[c * RPC:(c + 1) * RPC, :].T.astype(f8)   # [8192, 1024]
        # pack [N, RPC] -> [NC2, 128, 2*1024]: j = 256*cc + 128*k + p
        atp_np = np.ascontiguousarray(
            atb.reshape(NC2, 2, P, RPC).transpose(0, 2, 1, 3).reshape(NC2, P, 2 * RPC)
        )
        in_maps.append({"at": atp_np, "xt": xt, "wt": wt, "awr": awr})
    return in_maps


def kernel_with_results(X, A, W, a_w, trace=False):
    in_maps = make_in_maps(X, A, W, a_w)
    res = run_bass_kernel_spmd(_get_nc(), in_maps, list(range(NCORES)), trace=trace)
    out = np.concatenate(
        [np.ascontiguousarray(r["out"].T) for r in res.results], axis=0
    )
    return out.astype(np.float32), res


def kernel(X, A, W, a_w):
    out, _ = kernel_with_results(X, A, W, a_w)
    return out
